# revision 1
# baseline (speedup 1.0000x reference)
"""Trainium2 Bass kernel for nn_MultiHeadAttentionQuantum.

Math (verified vs reference to ~6e-7 rel):
  - _qlayer(x, phi)[t, w] reduces to prefix products of cos(x+phi):
      out[t, w]   = prod_{j<=w} cos(x[t,j]+phi[j])   (w >= 1)
      out[t, 0]   = prod_{j=1..7} cos(x[t,j]+phi[j])
    (RX rotations + CNOT-ring = prefix-XOR => Z-expectations = cos products.)
  - QuantumKernel sim factorizes rank-16:
      sim[i,j] = prod_{w<4} cos((q_iw - k_jw)/2) = F_i . G_j,
      F_m = prod_w (cos(q_w/2) or sin(q_w/2)) by bits of m; same for G with k.
    q, k in [-1, 1] => (q-k)/2 in [-1, 1] => every cos factor > 0 => the
    reference's abs() is a no-op.
  - softmax without max-subtraction (sim in [0,1], exp in [1, e]):
      acc = E @ [v | 1]  -> rows 0..7 numerator, row 8 denominator;
      a final 9x9 matmul applies W and adds bias via the b*den trick;
      the division happens in token-major layout.

Sharding: data-parallel over batch B=8, one batch per NeuronCore, no
collectives. Full inputs in, full output out; host only slices/stacks.

Layout per core ("linear split"): SBUF partition p holds tokens
16p..16p+15 (contiguous 512B DMA lines both directions). Token group
a = {16p+a : p} is a column slice everywhere, so the internal
key/query permutation is self-consistent and cancels out.

Matmuls run in float32r (4x faster than fp32 on the TRN2 PE; operands
are rounded to r-precision, ~2.5e-4 rel). The rounding enters via the
attention weights and averages out over 2048 keys; measured end-to-end
error vs the reference is ~3e-5 relative.
"""
import os
import numpy as np

import concourse.bass as bass
import concourse.tile as tile
from concourse import bacc, mybir
from concourse.bass_utils import run_bass_kernel_spmd
from concourse.masks import make_identity

F32 = mybir.dt.float32
F32R = mybir.dt.float32r
AL = mybir.AluOpType
ACTF = mybir.ActivationFunctionType

B, S, E = 8, 2048, 8
P = 128          # SBUF partitions
G = 16           # token groups per partition (S / P)
NF = 16          # feature rank
MAGIC = 12582912.0           # 1.5 * 2**23: fp32 round-to-nearest trick
TWO_PI = float(2.0 * np.pi)
HALF_PI = float(0.5 * np.pi)
MM2_RESIDUAL = os.environ.get("MM2_RESIDUAL", "0") == "1"
if os.environ.get("MM_DTYPE", "f32r") == "f32":
    F32R = F32

_NC_CACHE = {}


def _cos_chain(nc, work, x_bc, phi_bc, n, tagp, ag=G):
    """cos(x + phi) for n stacked phi-chains over ag token groups.
    x_bc / phi_bc: [P, n, ag, E] views (stride-0 broadcasts allowed).
    Returns c tile [P, n*ag*E]."""
    W = n * ag * E
    psi = work.tile([P, W], F32, tag=f"psi{tagp}")
    nc.vector.tensor_tensor(
        psi[:].rearrange("p (n a w) -> p n a w", n=n, a=ag), x_bc, phi_bc,
        op=AL.add)
    # range-reduce psi to [-pi, pi]:  psi - 2pi*round(psi/2pi)
    t1 = work.tile([P, W], F32, tag=f"t1{tagp}")
    nc.vector.tensor_scalar(t1[:], psi[:], float(1.0 / TWO_PI), MAGIC,
                            op0=AL.mult, op1=AL.add)
    t2 = work.tile([P, W], F32, tag=f"t2{tagp}")
    nc.vector.tensor_scalar(t2[:], t1[:], MAGIC, TWO_PI,
                            op0=AL.subtract, op1=AL.mult)
    red = work.tile([P, W], F32, tag=f"red{tagp}")
    nc.vector.tensor_tensor(red[:], psi[:], t2[:], op=AL.subtract)
    c = work.tile([P, W], F32, tag=f"c{tagp}")
    nc.scalar.activation(c[:], red[:], ACTF.Sin)  # sin(x+phi+pi/2)=cos(x+phi)
    return c


def _prefix_products(nc, work, c, n, tagp, ag=G):
    """u[j] = c[j-1]*c[j] (j>=1, u[0]=c[0]); v[j] = prod c[max(0,j-3)..j],
    per chain/group. c: [P, n*ag*8]."""
    c3 = c[:].rearrange("p (n a w) -> p n a w", n=n, a=ag)
    u = work.tile([P, n * ag * 8], F32, tag=f"u{tagp}")
    u3 = u[:].rearrange("p (n a w) -> p n a w", n=n, a=ag)
    nc.vector.tensor_copy(u3[:, :, :, 0:1], c3[:, :, :, 0:1])
    nc.vector.tensor_tensor(u3[:, :, :, 1:8], c3[:, :, :, 1:8],
                            c3[:, :, :, 0:7], op=AL.mult)
    v = work.tile([P, n * ag * 8], F32, tag=f"v{tagp}")
    v3 = v[:].rearrange("p (n a w) -> p n a w", n=n, a=ag)
    nc.vector.tensor_copy(v3[:, :, :, 0:2], u3[:, :, :, 0:2])
    nc.vector.tensor_tensor(v3[:, :, :, 2:8], u3[:, :, :, 2:8],
                            u3[:, :, :, 0:6], op=AL.mult)
    return u3, v3


def _suffix1(nc, work, u3, c3, out1, n, tagp, ag=G):
    """out1 [P, n, ag, 1] <- prod c[1..7] = u2*u4*u6*c7."""
    ta = work.tile([P, n * ag], F32, tag=f"s1a{tagp}")
    ta3 = ta[:].rearrange("p (n a) -> p n a", n=n).unsqueeze(3)
    nc.vector.tensor_tensor(ta3, u3[:, :, :, 2:3], u3[:, :, :, 4:5],
                            op=AL.mult)
    tb = work.tile([P, n * ag], F32, tag=f"s1b{tagp}")
    tb3 = tb[:].rearrange("p (n a) -> p n a", n=n).unsqueeze(3)
    nc.vector.tensor_tensor(tb3, u3[:, :, :, 6:7], c3[:, :, :, 7:8],
                            op=AL.mult)
    nc.vector.tensor_tensor(out1, ta3, tb3, op=AL.mult)


def _build_nc(reps=1):
    nc = bacc.Bacc("TRN2", target_bir_lowering=False, debug=False,
                   num_devices=B)
    x_d = nc.dram_tensor("x", [S, E], F32, kind="ExternalInput").ap()
    w9_d = nc.dram_tensor("w9", [9, 9], F32, kind="ExternalInput").ap()
    phis_d = nc.dram_tensor("phis", [3, E], F32, kind="ExternalInput").ap()
    out_d = nc.dram_tensor("out", [S, E], F32, kind="ExternalOutput").ap()

    with tile.TileContext(nc) as tc:
        with (
            tc.tile_pool(name="sb", bufs=1) as sb,
            tc.tile_pool(name="work", bufs=2) as work,
            tc.tile_pool(name="epool", bufs=4) as epool,
            tc.tile_pool(name="psb", bufs=3, space="PSUM") as psb,
            tc.tile_pool(name="psa", bufs=1, space="PSUM") as psa,
        ):
          for _rep in range(reps):
            # ---- loads & constants ----
            phib = sb.tile([P, 3 * E], F32, tag="phib")
            nc.sync.dma_start(
                phib[:],
                phis_d.rearrange("n w -> (n w)").unsqueeze(0)
                .broadcast_to((P, 3 * E)))
            x_sb = sb.tile([P, P], F32, tag="x")
            nc.sync.dma_start(
                x_sb[:], x_d.rearrange("(p a) w -> p (a w)", p=P))
            w9_sb = sb.tile([9, 9], F32, tag="w9")
            nc.sync.dma_start(w9_sb[:], w9_d[:])
            ident = sb.tile([P, P], F32, tag="ident")
            make_identity(nc, ident[:])
            half_pi = sb.tile([P, 1], F32, tag="half_pi_const")
            nc.vector.memset(half_pi[:], HALF_PI)
            phibs = sb.tile([P, 3 * E], F32, tag="phibs")
            nc.vector.tensor_scalar(phibs[:], phib[:], HALF_PI, None,
                                    op0=AL.add)
            phibs3 = phibs[:].rearrange("p (n w) -> p n w", n=3)

            # ---- PE warm-up: dummy transposes while DVE runs the
            # front-end chain (keeps the PE p-state/HAM at full clock) ----
            pewarm = psb.tile([P, P], F32, tag="small", bufs=2)
            for _ in range(int(os.environ.get('PEWARM', '26'))):
                nc.tensor.transpose(pewarm[:], ident[:], ident[:])

            # ---- fused q+k qlayer + features, emitted in two group
            # slices: a narrow chain (groups 0-3) unblocks the first
            # matmuls ~6us earlier; the rest overlaps the early loop ----
            x3 = x_sb[:].rearrange("p (a w) -> p a w", a=G)
            z4 = sb.tile([P, 2 * G * 4], F32, tag="z4")
            z44 = z4[:].rearrange("p (n a w) -> p n a w", n=2, a=G)
            feats = sb.tile([P, 2 * G * NF], F32, tag="feats")
            feats5 = feats[:].rearrange("p (n a hi lo) -> p n a hi lo",
                                        n=2, a=G, hi=4)

            def emit_front(a0, a1, tg, c=None):
                ag = a1 - a0
                if c is None:
                    x_bc = x3[:, a0:a1, :].unsqueeze(1).broadcast_to(
                        (P, 2, ag, E))
                    phiqk = phibs3[:, 0:2, :].unsqueeze(2).broadcast_to(
                        (P, 2, ag, E))
                    c = _cos_chain(nc, work, x_bc, phiqk, 2, tg, ag)
                c3 = c[:].rearrange("p (n a w) -> p n a w", n=2, a=ag)
                u3, v3 = _prefix_products(nc, work, c, 2, tg, ag)
                zs = z44[:, :, a0:a1, :]
                nc.vector.tensor_copy(zs[:, :, :, 1:4], v3[:, :, :, 1:4])
                _suffix1(nc, work, u3, c3, zs[:, :, :, 0:1], 2, tg, ag)
                # cs: [P, (b, n, a, w)]: b=0 cos(z/2), b=1 sin(z/2)
                cs = work.tile([P, 2 * 2 * ag * 4], F32, tag=f"cs{tg}")
                cs5 = cs[:].rearrange("p (b n a w) -> p b n a w",
                                      b=2, n=2, a=ag)
                nc.scalar.activation(cs5[:, 0], zs, ACTF.Sin,
                                     bias=half_pi[:], scale=0.5)
                nc.scalar.activation(cs5[:, 1], zs, ACTF.Sin, scale=0.5)

                def sel(w):
                    return cs5[:, :, :, :, w:w + 1].squeeze(4).transpose(
                        [0, 2, 3, 1])

                a01 = work.tile([P, 2 * ag * 4], F32, tag=f"a01{tg}")
                nc.vector.tensor_tensor(
                    a01[:].rearrange("p (n a b1 b0) -> p n a b1 b0",
                                     n=2, a=ag, b1=2),
                    sel(0).unsqueeze(3).broadcast_to((P, 2, ag, 2, 2)),
                    sel(1).unsqueeze(4).broadcast_to((P, 2, ag, 2, 2)),
                    op=AL.mult)
                a23 = work.tile([P, 2 * ag * 4], F32, tag=f"a23{tg}")
                nc.vector.tensor_tensor(
                    a23[:].rearrange("p (n a b3 b2) -> p n a b3 b2",
                                     n=2, a=ag, b3=2),
                    sel(2).unsqueeze(3).broadcast_to((P, 2, ag, 2, 2)),
                    sel(3).unsqueeze(4).broadcast_to((P, 2, ag, 2, 2)),
                    op=AL.mult)
                nc.vector.tensor_tensor(
                    feats5[:, :, a0:a1, :, :],
                    a01[:].rearrange("p (n a lo) -> p n a lo", n=2, a=ag)
                          .unsqueeze(3).broadcast_to((P, 2, ag, 4, 4)),
                    a23[:].rearrange("p (n a hi) -> p n a hi", n=2, a=ag)
                          .unsqueeze(4).broadcast_to((P, 2, ag, 4, 4)),
                    op=AL.mult)

            x_bc = x3.unsqueeze(1).broadcast_to((P, 2, G, E))
            phiqk = phibs3[:, 0:2, :].unsqueeze(2).broadcast_to((P, 2, G, E))
            c_qk = _cos_chain(nc, work, x_bc, phiqk, 2, "A", G)
            xv = x3.unsqueeze(1).broadcast_to((P, 1, G, E))
            phiv = phibs3[:, 2:3, :].unsqueeze(2).broadcast_to((P, 1, G, E))
            cv = _cos_chain(nc, work, xv, phiv, 1, "v")
            emit_front(0, 16, "A", c_qk)
            featv = feats[:].rearrange("p (n am) -> p n am", n=2)

            # ---- transpose features to [16, 2048] (PE transpose, packed) --
            # Emitted lazily: only the blocks the first matmuls need come
            # first; the rest interleave into the kt loop (PE gap filler).
            Ffeat = sb.tile([NF, S], F32R, tag="Ffeat")
            Gfeat = sb.tile([NF, S], F32R, tag="Gfeat")
            _tp_state = {"alt": 0}

            def emit_tp_block(ni, dst, blk):
                tf = psb.tile([NF, 512], F32, tag="small", bufs=2,
                              name=f"tf{ni}{blk}")
                for j in range(4):
                    a = blk * 4 + j
                    nc.tensor.transpose(
                        tf[:, j * P:(j + 1) * P],
                        featv[:, ni, a * NF:(a + 1) * NF], ident[:])
                # alternate DVE / ACT for the PSUM->SBUF copies
                nc.vector.tensor_copy(
                    dst[:, blk * 512:(blk + 1) * 512], tf[:])
                _tp_state["alt"] += 1

            def emit_tp_group(a):
                # single token-group transpose for G (one group per kt)
                tg = psb.tile([NF, P], F32, tag="small", bufs=2,
                              name=f"tg{a}")
                nc.tensor.transpose(tg[:], featv[:, 1, a * NF:(a + 1) * NF],
                                    ident[:])
                nc.vector.tensor_copy(Gfeat[:, a * P:(a + 1) * P], tg[:])

            emit_tp_block(0, Ffeat, 0)
            emit_tp_group(0)
            emit_tp_block(0, Ffeat, 1)

            # ---- qlayer for v (cos precomputed; overlaps loop start) ----
            cv3 = cv[:].rearrange("p (n a w) -> p n a w", n=1, a=G)
            uv3, vv3 = _prefix_products(nc, work, cv, 1, "v")
            vaug = sb.tile([P, G * 9], F32, tag="vaug")
            nc.vector.memset(vaug[:], 1.0)          # col 8 of each group = 1
            va4 = vaug[:].rearrange("p (a w) -> p a w", a=G).unsqueeze(1)
            nc.vector.tensor_copy(va4[:, :, :, 1:4], vv3[:, :, :, 1:4])
            nc.vector.tensor_tensor(va4[:, :, :, 4:8], vv3[:, :, :, 4:8],
                                    vv3[:, :, :, 0:4], op=AL.mult)
            _suffix1(nc, work, uv3, cv3, va4[:, :, :, 0:1], 1, "v")
            vaug_r = sb.tile([P, G * 9], F32R, tag="vaug_r")
            nc.vector.tensor_copy(vaug_r[:], vaug[:])
            vts = [vaug_r]
            if MM2_RESIDUAL:
                vaug_e = sb.tile([P, G * 9], F32R, tag="vaug_e")
                nc.vector.tensor_tensor(vaug_e[:], vaug[:], vaug_r[:],
                                        op=AL.subtract)
                vts.append(vaug_e)

            # ---- main loop: 2 query half-passes, pipelined over kt ----
            ftok = sb.tile([P, G * 9], F32, tag="ftok")
            recip = sb.tile([P, G], F32, tag="recip")
            outt = sb.tile([P, P], F32, tag="outt")
            out_v = out_d.rearrange("(p a) w -> p (a w)", p=P)

            pending_tail = [None]

            for hp in range(2):          # query half-pass (1024 queries)
                q0 = hp * 1024
                acc = psa.tile([9, 1024], F32, tag="acc")
                esbs = {}
                for kt in range(G + 2):
                    if hp == 0 and 1 <= kt < G:
                        emit_tp_group(kt)
                    if hp == 0 and kt in (11, 13):
                        emit_tp_block(0, Ffeat, {11: 2, 13: 3}[kt])
                    if hp == 1 and kt == 4 and pending_tail[0] is not None:
                        pending_tail[0]()    # pass-0 tail, amortized here
                        pending_tail[0] = None
                    if kt < G:
                        eps = psb.tile([P, 1024], F32, tag="big", bufs=2)
                        for j in range(2):
                            nc.tensor.matmul(
                                eps[:, j * 512:(j + 1) * 512],
                                Gfeat[:, kt * P:(kt + 1) * P],
                                Ffeat[:, q0 + j * 512:q0 + (j + 1) * 512],
                                start=True, stop=True)
                        esb = epool.tile([P, 1024], F32R, tag="e", bufs=6)
                        if kt == 0:
                            # split: lets the exp stream start ~1us earlier
                            nc.scalar.activation(esb[:, 0:512],
                                                 eps[:, 0:512], ACTF.Exp)
                            nc.scalar.activation(esb[:, 512:1024],
                                                 eps[:, 512:1024], ACTF.Exp)
                        else:
                            nc.scalar.activation(esb[:], eps[:], ACTF.Exp)
                        esbs[kt] = esb
                    if kt >= 2:
                        kp = kt - 2
                        esb = esbs.pop(kp)
                        for j in range(2):
                            for vi, vt in enumerate(vts):
                                nc.tensor.matmul(
                                    acc[:, j * 512:(j + 1) * 512],
                                    vt[:, kp * 9:(kp + 1) * 9],
                                    esb[:, j * 512:(j + 1) * 512],
                                    start=(kp == 0 and vi == 0),
                                    stop=(kp == G - 1
                                          and vi == len(vts) - 1))

                # ---- tail for this half; pass-0's is deferred into the
                # middle of pass-1's loop so it doesn't stall the exp stream
                def make_tail(hp, acc, fin_tag="small", act_copy=False):
                    def emit():
                        numden = sb.tile([9, 1024], F32, tag="numden",
                                         bufs=2, name=f"numden{hp}")
                        tailt = psb.tile([P, 8 * 9], F32, tag="small",
                                         bufs=2, name=f"tailt{hp}")
                        for j in range(2):
                            if act_copy:
                                nc.scalar.copy(
                                    numden[:, j * 512:(j + 1) * 512],
                                    acc[:, j * 512:(j + 1) * 512])
                            else:
                                nc.vector.tensor_copy(
                                    numden[:, j * 512:(j + 1) * 512],
                                    acc[:, j * 512:(j + 1) * 512])
                            fin_ps = psb.tile([9, 512], F32, tag=fin_tag,
                                              bufs=2, name=f"finps{hp}{j}")
                            nc.tensor.matmul(
                                fin_ps[:], w9_sb[:],
                                numden[:, j * 512:(j + 1) * 512],
                                start=True, stop=True)
                            fin_sb = sb.tile([9, 512], F32, tag="fin",
                                             bufs=2, name=f"finsb{hp}{j}")
                            nc.vector.tensor_copy(fin_sb[:], fin_ps[:])
                            for aa in range(4):
                                a = j * 4 + aa
                                nc.tensor.transpose(
                                    tailt[:, a * 9:(a + 1) * 9],
                                    fin_sb[:, aa * P:(aa + 1) * P],
                                    ident[0:9, 0:9])
                        hs = slice(hp * 72, hp * 72 + 72)
                        nc.vector.tensor_copy(ftok[:, hs], tailt[:])
                        ft3 = ftok[:].rearrange("p (a e) -> p a e", a=G)
                        a0 = hp * 8
                        nc.vector.reciprocal(
                            recip[:, a0:a0 + 8].unsqueeze(2),
                            ft3[:, a0:a0 + 8, 8:9])
                        ot3 = outt[:].rearrange("p (a e) -> p a e", a=G)
                        nc.vector.tensor_tensor(
                            ot3[:, a0:a0 + 8, :], ft3[:, a0:a0 + 8, 0:8],
                            recip[:, a0:a0 + 8].unsqueeze(2)
                            .broadcast_to((P, 8, E)), op=AL.mult)
                        nc.sync.dma_start(
                            out_v[:, hp * 64:hp * 64 + 64],
                            outt[:, hp * 64:hp * 64 + 64])
                    return emit

                if hp == 0:
                    pending_tail[0] = make_tail(hp, acc)
                else:
                    make_tail(hp, acc, fin_tag="big")()

    nc.compile()
    return nc


def get_nc(reps=1):
    if reps not in _NC_CACHE:
        _NC_CACHE[reps] = _build_nc(reps)
    return _NC_CACHE[reps]


def kernel(x, phi_q, phi_k, phi_v, W, b, **_unused):
    x = np.ascontiguousarray(np.asarray(x, dtype=np.float32))
    W = np.asarray(W, dtype=np.float32)
    bb = np.asarray(b, dtype=np.float32)
    w9 = np.zeros((9, 9), np.float32)
    w9[0:8, 0:8] = W.T          # lhsT[d, e] = W[e, d]
    w9[8, 0:8] = bb             # bias enters as b * den
    w9[8, 8] = 1.0              # denominator passthrough
    phis = np.stack([phi_q, phi_k, phi_v]).astype(np.float32)

    nc = get_nc()
    in_maps = [{"x": x[i], "w9": w9, "phis": phis} for i in range(B)]
    res = run_bass_kernel_spmd(nc, in_maps, list(range(B)))
    return np.stack([res.results[i]["out"] for i in range(B)])

